# revision 9
# baseline (speedup 1.0000x reference)
"""LightGCN (AIM variant) distributed Bass kernel for 8 TRN2 NeuronCores.

Strategy (destination sharding):
  - 150000 nodes split into 8 slices of 18750 (padded to 18816 = 147*128 rows
    per slice; pad rows are always zero). Core k owns the destinations of
    slice k and all edges pointing into them (~500k edges/core).
  - z-substitution: with z = dis * x (dis = deg^-1/2), each LightGCN layer is
    x_{l+1}[c] = dis[c] * sum_{e in(c)} z_l[row_e]  -- a pure gather +
    segment-sum; the per-edge norm multiplier disappears.
  - z tables are bf16 [150528, 128]: each row's first 64 cols hold the
    embedding, the rest is never read (present only to satisfy the 256B
    dma_gather element granularity).
  - Per layer: each core gathers z rows for its edges (dma_gather custom
    instruction, 4 SWDGE queues, int16 indices -> the table is split into 5
    chunks of <=30112 rows), segment-sums them on the TensorEngine via
    on-device-built one-hot bf16 matrices (PSUM accumulation per
    128-destination group, 4 groups packed per PSUM bank), scales by dis,
    and AllGathers the new z slices.
  - Epilogue: item rows L2-normalized (*1.5), final out table AllGathered,
    the 2x8192 label endpoints gathered, ranks + beta terms computed on
    device, and the (4096, 4096) outer-sum outputs written (rows sharded
    across cores).
"""

import numpy as np

import concourse.bass as bass
import concourse.bacc as bacc
import concourse.tile as tile
import concourse.mybir as mybir
from concourse.bass_utils import run_bass_kernel_spmd
from concourse.masks import make_identity

# ---------------------------------------------------------------- constants
N = 150000
D = 64
ZW = 128             # z-table row width (bf16) = 256B
NLAYERS = 3
NLAB = 8192
NCORE = 8
SLICE_REAL = 18750
SLICE_PAD = 18816    # 147 * 128
G_GROUPS = 147
SUPER = 4            # dest groups per super-group / PSUM bank
TABLE = NCORE * SLICE_PAD   # 150528
CHUNK = 30112
NCHUNK = 5
INSTR_MAX = 4096     # idxs per dma_gather instruction (single_packet=False)
IDXW = 4096          # idx window width in int16 columns (= 65536 idxs)
BETA_WIN = (N + 63) // 64   # 2344 windows of 64 floats
EPS = 1e-12

F32 = mybir.dt.float32
BF16 = mybir.dt.bfloat16
I16 = mybir.dt.int16


def _chunk_bounds():
    lo = [c * CHUNK for c in range(NCHUNK)]
    hi = [min((c + 1) * CHUNK, TABLE) for c in range(NCHUNK)]
    return lo, hi


def _zrow_locals():
    lo, hi = _chunk_bounds()
    out = []
    for c in range(NCHUNK):
        z = None
        for s in range(NCORE):
            zr = s * SLICE_PAD + SLICE_REAL
            if lo[c] <= zr < hi[c]:
                z = zr - lo[c]
                break
        assert z is not None
        out.append(z)
    return out


def _wrap16(flat):
    w = flat.reshape(-1, 16).T
    return np.tile(w, (8, 1)).copy()


# ---------------------------------------------------------------- host prep
def _prep(emb_weight, beta_weight, alpha, edge_index, edge_label_index,
          num_users, scaling_factor):
    row = np.asarray(edge_index[0]).astype(np.int64)
    col = np.asarray(edge_index[1]).astype(np.int64)
    num_users = int(num_users)

    deg = np.bincount(col, minlength=N).astype(np.int64)
    r_tab = (row // SLICE_REAL) * SLICE_PAD + (row % SLICE_REAL)
    core_of = col // SLICE_REAL
    zrl = _zrow_locals()

    NSG = (G_GROUPS + SUPER - 1) // SUPER     # 37 super-groups
    sg_of_g = np.arange(G_GROUPS) // SUPER

    # cell order: for sg, for chunk, for g in sg
    cell_rank = np.full((G_GROUPS, NCHUNK), -1, np.int64)
    cells = []               # rank -> (g, c)
    for sg in range(NSG):
        gs = range(sg * SUPER, min((sg + 1) * SUPER, G_GROUPS))
        for c in range(NCHUNK):
            for g in gs:
                cell_rank[g, c] = len(cells)
                cells.append((g, c))
    NCELL = len(cells)

    per_core = []
    counts = np.zeros((NCORE, NCELL), np.int64)
    for k in range(NCORE):
        m = core_of == k
        ld = col[m] - k * SLICE_REAL
        rt = r_tab[m]
        ch = rt // CHUNK
        g = ld // 128
        crk = cell_rank[g, ch]
        order = np.argsort(crk, kind="stable")
        ld, rt, crk = ld[order], rt[order], crk[order]
        counts[k] = np.bincount(crk, minlength=NCELL)
        per_core.append((ld, rt, crk))

    P = ((counts.max(axis=0) + 127) // 128) * 128        # [NCELL]
    cell_start = np.zeros(NCELL + 1, np.int64)
    cell_start[1:] = np.cumsum(P)
    S = int(cell_start[-1])
    SCOLS = S // 128
    assert P.max() <= INSTR_MAX, P.max()

    # per-128-block group id + first/last flags
    blk_g = np.empty(SCOLS, np.int64)
    for r, (g, c) in enumerate(cells):
        blk_g[cell_start[r] // 128: cell_start[r + 1] // 128] = g
    g_first_blk = np.full(G_GROUPS, -1, np.int64)
    g_last_blk = np.full(G_GROUPS, -1, np.int64)
    for b in range(SCOLS):
        g = blk_g[b]
        if g_first_blk[g] < 0:
            g_first_blk[g] = b
        g_last_blk[g] = b

    # instruction list: greedy fill from consecutive cells sharing (sg, chunk)
    instrs = []
    col_cur = 0
    r = 0
    while r < NCELL:
        g0, c0 = cells[r]
        sg0 = int(sg_of_g[g0])
        soff = int(cell_start[r])
        n = 0
        while r < NCELL and n + int(P[r]) <= INSTR_MAX:
            g, c = cells[r]
            if c != c0 or int(sg_of_g[g]) != sg0:
                break
            n += int(P[r])
            r += 1
        assert n > 0
        ncols = n // 16
        if col_cur % IDXW + ncols > IDXW:
            col_cur = (col_cur // IDXW + 1) * IDXW
        blocks = []
        for b in range(soff // 128, (soff + n) // 128):
            g = int(blk_g[b])
            blocks.append((b - soff // 128, g,
                           bool(b == g_first_blk[g]), bool(b == g_last_blk[g])))
        instrs.append(dict(c=c0, n=n, icol=col_cur, soff=soff, sg=sg0,
                           blocks=blocks))
        col_cur += ncols
    assert sum(i["n"] for i in instrs) == S
    TOTCOLS = ((col_cur + IDXW - 1) // IDXW) * IDXW

    slot2idx = np.empty(S, np.int64)
    for i in instrs:
        slot2idx[i["soff"]:i["soff"] + i["n"]] = np.arange(
            i["icol"] * 16, i["icol"] * 16 + i["n"])

    lab_src = np.asarray(edge_label_index[0]).astype(np.int64)
    lab_dst = np.asarray(edge_label_index[1]).astype(np.int64)
    lab_nodes = np.concatenate([lab_src, lab_dst])
    lab_tab = (lab_nodes // SLICE_REAL) * SLICE_PAD + (lab_nodes % SLICE_REAL)

    alpha = np.asarray(alpha, np.float32).reshape(-1)
    emb_weight = np.asarray(emb_weight, np.float32)
    beta_flat = np.zeros(BETA_WIN * 64, np.float32)
    beta_flat[:N] = np.asarray(beta_weight, np.float32).reshape(-1)

    chunk_of_cell = np.array([c for _, c in cells], np.int64)

    in_maps = []
    for k in range(NCORE):
        ld, rt, crk = per_core[k]
        nk = ld.shape[0]
        first_idx = np.zeros(NCELL, np.int64)
        cnt = counts[k]
        first_idx[1:] = np.cumsum(cnt)[:-1]
        pos_in_cell = np.arange(nk) - np.repeat(first_idx, cnt)
        slot = cell_start[crk] + pos_in_cell

        idx_flat = np.zeros(TOTCOLS * 16, np.int16)
        for i in instrs:
            idx_flat[i["icol"] * 16: i["icol"] * 16 + i["n"]] = zrl[i["c"]]
        idx_flat[slot2idx[slot]] = (rt - chunk_of_cell[crk] * CHUNK).astype(np.int16)

        dsel = np.full(S, -1.0, np.float32)
        dsel[slot] = (ld % 128).astype(np.float32)

        degs = np.zeros(SLICE_PAD, np.float32)
        degs[:SLICE_REAL] = deg[k * SLICE_REAL:(k + 1) * SLICE_REAL]
        mdeg = (degs > 0).astype(np.float32)
        ids = np.arange(k * SLICE_REAL, k * SLICE_REAL + SLICE_PAD)
        mitem = ((ids >= num_users) &
                 (ids < k * SLICE_REAL + SLICE_REAL)).astype(np.float32)

        embs = np.zeros((SLICE_PAD, D), np.float32)
        embs[:SLICE_REAL] = emb_weight[k * SLICE_REAL:(k + 1) * SLICE_REAL]

        lab_parts = []
        for c in range(NCHUNK):
            v = np.where(lab_tab // CHUNK == c, lab_tab - c * CHUNK,
                         zrl[c]).astype(np.int16)
            lab_parts.append(_wrap16(v))
        lab_idx = np.concatenate(lab_parts, axis=1)      # [128, 5*1024]

        bwin = _wrap16((lab_dst // 64).astype(np.int16))  # [128, 512]
        e8 = np.zeros((NLAB, 64), np.float32)
        e8[np.arange(NLAB), lab_dst % 64] = 1.0
        e8 = e8.reshape(64, 128, 64).transpose(1, 0, 2).copy()

        osel = np.zeros((8, 64), np.float32)
        for t in range(4):
            osel[t, k * 4 + t] = 1.0
            osel[4 + t, 32 + k * 4 + t] = 1.0
        osel = np.tile(osel[None, :, :], (128, 1, 1)).copy()

        in_maps.append({
            "emb": embs,
            "degf": degs.reshape(G_GROUPS, 128).T.copy(),
            "mdeg": mdeg.reshape(G_GROUPS, 128).T.copy(),
            "mitem": mitem.reshape(G_GROUPS, 128).T.copy(),
            "alpha": np.tile(alpha.reshape(1, 4), (128, 1)),
            "scal": np.full((128, 1), float(scaling_factor), np.float32),
            "idx": _wrap16(idx_flat),
            "dsel": dsel.reshape(-1, 128).T.copy(),
            "lab": lab_idx,
            "bwin": bwin,
            "e8": e8.reshape(128, 64 * 64),
            "beta": beta_flat.reshape(BETA_WIN, 64),
            "iota": np.tile(np.arange(128, dtype=np.float32)[None, :], (128, 1)),
            "osel": osel.reshape(128, 8 * 64),
        })

    meta = dict(instrs=instrs, S=S, SCOLS=SCOLS, TOTCOLS=TOTCOLS, NSG=NSG)
    return in_maps, meta


# ---------------------------------------------------------------- builder
def _bc_mid(base_ap, p_count, mid_count, last_count):
    """[p, last] AP -> [p, mid(bcast), last]"""
    return bass.AP(base_ap.tensor, base_ap.offset,
                   [list(base_ap.ap[0])[:1] + [p_count],
                    [0, mid_count],
                    [1, last_count]])


def _build(meta):
    SCOLS = meta["SCOLS"]
    TOTCOLS = meta["TOTCOLS"]

    nc = bacc.Bacc(None, target_bir_lowering=False, num_swdge_queues=4)
    dp = nc.declare_dram_parameter
    emb_e = dp("emb", [SLICE_PAD, D], F32, isOutput=False)
    degf_e = dp("degf", [128, G_GROUPS], F32, isOutput=False)
    mdeg_e = dp("mdeg", [128, G_GROUPS], F32, isOutput=False)
    mitem_e = dp("mitem", [128, G_GROUPS], F32, isOutput=False)
    alpha_e = dp("alpha", [128, 4], F32, isOutput=False)
    scal_e = dp("scal", [128, 1], F32, isOutput=False)
    idx_e = dp("idx", [128, TOTCOLS], I16, isOutput=False)
    dsel_e = dp("dsel", [128, SCOLS], F32, isOutput=False)
    lab_e = dp("lab", [128, NCHUNK * 1024], I16, isOutput=False)
    bwin_e = dp("bwin", [128, 512], I16, isOutput=False)
    e8_e = dp("e8", [128, 64 * 64], F32, isOutput=False)
    beta_e = dp("beta", [BETA_WIN, 64], F32, isOutput=False)
    iota_e = dp("iota", [128, 128], F32, isOutput=False)
    osel_e = dp("osel", [128, 8 * 64], F32, isOutput=False)
    out_e = dp("out", [2, 512, 4096], F32, isOutput=True)

    zslice = [nc.dram_tensor(f"zs{l}", [SLICE_PAD, ZW], BF16)
              for l in range(NLAYERS)]
    zfull = [nc.dram_tensor(f"zf{l}", [TABLE, ZW], BF16, addr_space="Shared")
             for l in range(NLAYERS)]
    oslice = nc.dram_tensor("oslice", [SLICE_PAD, D], F32)
    ofull = nc.dram_tensor("ofull", [TABLE, D], F32, addr_space="Shared")
    rrow_d = nc.dram_tensor("rrow", [1, NLAB], F32)

    with tile.TileContext(nc) as tc:
        with (
            tc.tile_pool(name="persist", bufs=1) as pp,
            tc.tile_pool(name="psum", bufs=4, space="PSUM") as psp,
        ):
            out_acc = pp.tile([128, G_GROUPS, D], F32)
            dis = pp.tile([128, G_GROUPS], F32)
            alpha_sb = pp.tile([128, 4], F32)
            scal_sb = pp.tile([128, 1], F32)
            mitem_sb = pp.tile([128, G_GROUPS], F32)

            nc.sync.dma_start(out=alpha_sb[:], in_=alpha_e[:, :])
            nc.sync.dma_start(out=scal_sb[:], in_=scal_e[:, :])
            nc.sync.dma_start(out=mitem_sb[:], in_=mitem_e[:, :])
            _layers(nc, tc, meta, locals())
            _epilogue(nc, tc, meta, locals())
    return nc


def _layers(nc, tc, meta, env):
    instrs = meta["instrs"]
    SCOLS = meta["SCOLS"]
    clo, chi = _chunk_bounds()
    mul = mybir.AluOpType.mult
    out_acc = env["out_acc"]; dis = env["dis"]; alpha_sb = env["alpha_sb"]
    psp = env["psp"]
    emb_e = env["emb_e"]; degf_e = env["degf_e"]; mdeg_e = env["mdeg_e"]
    dsel_e = env["dsel_e"]; iota_e = env["iota_e"]; idx_e = env["idx_e"]
    zslice = env["zslice"]; zfull = env["zfull"]
    rg = [list(range(NCORE))]
    with (
        tc.tile_pool(name="work", bufs=3) as wp,
        tc.tile_pool(name="gat", bufs=6) as gp,
        tc.tile_pool(name="bmat", bufs=8) as bp,
        tc.tile_pool(name="idxw", bufs=2) as ip,
        tc.tile_pool(name="lpersist", bufs=1) as lp,
    ):
            dsel_sb = lp.tile([128, SCOLS], F32)
            iota_sb = lp.tile([128, 128], F32)
            nc.sync.dma_start(out=dsel_sb[:], in_=dsel_e[:, :])
            nc.sync.dma_start(out=iota_sb[:], in_=iota_e[:, :])

            # ---- dis = (deg > 0) / sqrt(max(deg, 1))
            degf = wp.tile([128, G_GROUPS], F32, tag="deg")
            mdeg = wp.tile([128, G_GROUPS], F32, tag="deg")
            nc.sync.dma_start(out=degf[:], in_=degf_e[:, :])
            nc.sync.dma_start(out=mdeg[:], in_=mdeg_e[:, :])
            degc = wp.tile([128, G_GROUPS], F32, tag="deg")
            nc.vector.tensor_scalar_max(out=degc[:], in0=degf[:], scalar1=1.0)
            dsq = wp.tile([128, G_GROUPS], F32, tag="deg")
            nc.scalar.activation(out=dsq[:], in_=degc[:],
                                 func=mybir.ActivationFunctionType.Sqrt)
            drc = wp.tile([128, G_GROUPS], F32, tag="deg")
            nc.vector.reciprocal(out=drc[:], in_=dsq[:])
            nc.vector.tensor_mul(out=dis[:], in0=drc[:], in1=mdeg[:])

            # ---- z0 slice + out_acc init (hi/lo bf16 split of z)
            for g in range(G_GROUPS):
                et = wp.tile([128, D], F32, tag="emb")
                nc.sync.dma_start(out=et[:], in_=emb_e[g * 128:(g + 1) * 128, :])
                z032 = wp.tile([128, D], F32, tag="z032")
                nc.vector.tensor_scalar(out=z032[:], in0=et[:],
                                        scalar1=dis[:, g:g + 1], scalar2=None,
                                        op0=mul)
                zh = wp.tile([128, D], BF16, tag="zh")
                nc.vector.tensor_copy(out=zh[:], in_=z032[:])
                hi32 = wp.tile([128, D], F32, tag="hi32")
                nc.vector.tensor_copy(out=hi32[:], in_=zh[:])
                zl = wp.tile([128, D], BF16, tag="zl")
                nc.vector.tensor_tensor(out=zl[:], in0=z032[:], in1=hi32[:],
                                        op=mybir.AluOpType.subtract)
                nc.sync.dma_start(
                    out=zslice[0][g * 128:(g + 1) * 128, 0:D], in_=zh[:])
                nc.sync.dma_start(
                    out=zslice[0][g * 128:(g + 1) * 128, D:ZW], in_=zl[:])
                nc.vector.tensor_scalar(out=out_acc[:, g, :], in0=et[:],
                                        scalar1=alpha_sb[:, 0:1], scalar2=None,
                                        op0=mul)
            nc.gpsimd.collective_compute(
                "AllGather", mybir.AluOpType.bypass, replica_groups=rg,
                ins=[zslice[0].ap().opt()], outs=[zfull[0].ap().opt()])

            # ---- propagation layers
            qrr = 0
            for l in range(NLAYERS):
                zf = zfull[l]
                win_tile = None
                win_id = -1
                state = dict(sg=-1, psum=None, done=[], started=False)

                def flush_groups():
                    psum = state["psum"]
                    for g in state["done"]:
                        gl = g % SUPER
                        xs = wp.tile([128, D], F32, tag="xs")
                        ps_hl = bass.AP(psum.tensor, psum.offset + gl * ZW,
                                        [list(psum.ap[0])[:1] + [128],
                                         [1, D], [D, 2]])
                        nc.vector.reduce_sum(xs[:], ps_hl,
                                             axis=mybir.AxisListType.X)
                        if l < NLAYERS - 1:
                            zn32 = wp.tile([128, D], F32, tag="zn32")
                            nc.vector.tensor_scalar(
                                out=zn32[:], in0=xs[:],
                                scalar1=dis[:, g:g + 1],
                                scalar2=dis[:, g:g + 1], op0=mul, op1=mul)
                            zh = wp.tile([128, D], BF16, tag="zh")
                            nc.vector.tensor_copy(out=zh[:], in_=zn32[:])
                            hi32 = wp.tile([128, D], F32, tag="hi32")
                            nc.vector.tensor_copy(out=hi32[:], in_=zh[:])
                            zl = wp.tile([128, D], BF16, tag="zl")
                            nc.vector.tensor_tensor(
                                out=zl[:], in0=zn32[:], in1=hi32[:],
                                op=mybir.AluOpType.subtract)
                            nc.sync.dma_start(
                                out=zslice[l + 1][g * 128:(g + 1) * 128, 0:D],
                                in_=zh[:])
                            nc.sync.dma_start(
                                out=zslice[l + 1][g * 128:(g + 1) * 128, D:ZW],
                                in_=zl[:])
                        t2 = wp.tile([128, D], F32, tag="t2")
                        nc.vector.tensor_scalar(
                            out=t2[:], in0=xs[:],
                            scalar1=dis[:, g:g + 1],
                            scalar2=alpha_sb[:, l + 1:l + 2], op0=mul, op1=mul)
                        nc.vector.tensor_add(out=out_acc[:, g, :],
                                             in0=out_acc[:, g, :], in1=t2[:])
                    state["done"] = []

                for ins in instrs:
                    c, n, icol, soff, sg = (ins["c"], ins["n"], ins["icol"],
                                            ins["soff"], ins["sg"])
                    if sg != state["sg"]:
                        flush_groups()
                        state["psum"] = psp.tile([128, SUPER * ZW], F32, tag="acc", name="psacc")
                        state["sg"] = sg
                        state["started"] = False
                    psum = state["psum"]
                    nb = n // 128
                    w = icol // IDXW
                    if w != win_id:
                        win_tile = ip.tile([128, IDXW], I16, tag="idxw")
                        nc.sync.dma_start(
                            out=win_tile[:],
                            in_=idx_e[:, w * IDXW:(w + 1) * IDXW])
                        win_id = w
                    gt = gp.tile([128, INSTR_MAX // 128, ZW], BF16, tag="gat")
                    wc = icol % IDXW
                    nc.gpsimd.dma_gather(
                        out_ap=gt[:, :nb, :],
                        in_ap=zf[clo[c]:chi[c], :],
                        idxs_ap=win_tile[:, wc:wc + n // 16],
                        num_idxs=n,
                        num_idxs_reg=n,
                        elem_size=ZW,
                        single_packet=False,
                        queue_num=qrr % 4,
                    )
                    qrr += 1
                    col0 = soff // 128
                    for p0 in range(0, nb, 8):
                        pnb = min(8, nb - p0)
                        bt = bp.tile([128, 8, 128], BF16, tag="bmat")
                        in0 = dsel_sb[:, col0 + p0:col0 + p0 + pnb
                                      ].to_broadcast([128, pnb, 128])
                        in1 = _bc_mid(iota_sb[:], 128, pnb, 128)
                        nc.vector.tensor_tensor(out=bt[:, :pnb, :], in0=in0,
                                                in1=in1,
                                                op=mybir.AluOpType.is_equal)
                        for j in range(pnb):
                            jj = p0 + j
                            _, g, first, last = ins["blocks"][jj]
                            gl = g % SUPER
                            nc.tensor.matmul(
                                psum[:, gl * ZW:(gl + 1) * ZW],
                                bt[:, j, :], gt[:, jj, :],
                                start=not state["started"], stop=last,
                                skip_group_check=True)
                            state["started"] = True
                            if last:
                                state["done"].append(g)
                flush_groups()
                if l < NLAYERS - 1:
                    nc.gpsimd.collective_compute(
                        "AllGather", mybir.AluOpType.bypass, replica_groups=rg,
                        ins=[zslice[l + 1].ap().opt()],
                        outs=[zfull[l + 1].ap().opt()])


def _epilogue(nc, tc, meta, env):
    clo, chi = _chunk_bounds()
    mul = mybir.AluOpType.mult
    add = mybir.AluOpType.add
    out_acc = env["out_acc"]; dis = env["dis"]; scal_sb = env["scal_sb"]
    mitem_sb = env["mitem_sb"]; psp = env["psp"]
    oslice = env["oslice"]; ofull = env["ofull"]; rrow_d = env["rrow_d"]
    lab_e = env["lab_e"]; bwin_e = env["bwin_e"]; e8_e = env["e8_e"]
    beta_e = env["beta_e"]; osel_e = env["osel_e"]; out_e = env["out_e"]
    pp = env["pp"]
    rg = [list(range(NCORE))]
    with (
        tc.tile_pool(name="ep", bufs=1) as ep,
        tc.tile_pool(name="ew", bufs=3) as wp,
        tc.tile_pool(name="eg", bufs=2) as gp,
        tc.tile_pool(name="orow", bufs=2) as op,
    ):
            # ---- item normalization + out slice
            for g in range(G_GROUPS):
                sq = wp.tile([128, D], F32, tag="sq")
                nc.vector.tensor_mul(out=sq[:], in0=out_acc[:, g, :],
                                     in1=out_acc[:, g, :])
                ss = wp.tile([128, 1], F32, tag="ss")
                nc.vector.reduce_sum(ss[:], sq[:], axis=mybir.AxisListType.X)
                nrm = wp.tile([128, 1], F32, tag="ss")
                nc.scalar.activation(out=nrm[:], in_=ss[:],
                                     func=mybir.ActivationFunctionType.Sqrt)
                nc.vector.tensor_scalar_max(out=nrm[:], in0=nrm[:], scalar1=EPS)
                rec = wp.tile([128, 1], F32, tag="ss")
                nc.vector.reciprocal(out=rec[:], in_=nrm[:])
                fac = wp.tile([128, 1], F32, tag="ss")
                nc.vector.tensor_scalar(out=fac[:], in0=rec[:],
                                        scalar1=scal_sb[:, 0:1], scalar2=-1.0,
                                        op0=mul, op1=add)
                nc.vector.tensor_scalar(out=fac[:], in0=fac[:],
                                        scalar1=mitem_sb[:, g:g + 1], scalar2=1.0,
                                        op0=mul, op1=add)
                on = wp.tile([128, D], F32, tag="on")
                nc.vector.tensor_scalar(out=on[:], in0=out_acc[:, g, :],
                                        scalar1=fac[:, 0:1], scalar2=None,
                                        op0=mul)
                nc.sync.dma_start(out=oslice[g * 128:(g + 1) * 128, :], in_=on[:])
            nc.gpsimd.collective_compute(
                "AllGather", mybir.AluOpType.bypass, replica_groups=rg,
                ins=[oslice.ap().opt()], outs=[ofull.ap().opt()])

            # ---- label gathers (16384 slots, 5 chunk passes summed)
            lab_sb = ep.tile([128, NCHUNK * 1024], I16, tag="lab")
            nc.sync.dma_start(out=lab_sb[:], in_=lab_e[:, :])
            acc_lab = pp.tile([128, 128, D], F32, tag="out_acc")
            for c in range(NCHUNK):
                for half in range(2):
                    lt = gp.tile([128, 64, D], F32, tag="labg")
                    for piece in range(8):
                        pc = half * 8 + piece
                        nc.gpsimd.dma_gather(
                            out_ap=lt[:, piece * 8:(piece + 1) * 8, :],
                            in_ap=ofull[clo[c]:chi[c], :],
                            idxs_ap=lab_sb[:, c * 1024 + pc * 64:
                                           c * 1024 + (pc + 1) * 64],
                            num_idxs=1024,
                            num_idxs_reg=1024,
                            elem_size=D,
                            single_packet=True,
                            queue_num=pc % 4,
                        )
                    dst = acc_lab[:, half * 64:(half + 1) * 64, :]
                    if c == 0:
                        nc.vector.tensor_copy(out=dst, in_=lt[:])
                    else:
                        nc.vector.tensor_add(out=dst, in0=dst, in1=lt[:])

            # rank[i] at [i%128, i//128]; src slots g 0..63, dst slots g 64..127
            rmul = ep.tile([128, 64, D], F32, tag="e16a")
            nc.vector.tensor_mul(out=rmul[:], in0=acc_lab[:, :64, :],
                                 in1=acc_lab[:, 64:, :])
            rank = ep.tile([128, 64], F32, tag="rank")
            nc.vector.reduce_sum(rank[:], rmul[:], axis=mybir.AxisListType.X)

            # ---- beta windows
            bwin_sb = ep.tile([128, 512], I16, tag="bwin")
            nc.sync.dma_start(out=bwin_sb[:], in_=bwin_e[:, :])
            e8_sb = ep.tile([128, 64, D], F32, tag="e16b")
            nc.sync.dma_start(out=e8_sb[:],
                              in_=e8_e[:, :].rearrange("p (g d) -> p g d", d=D))
            bw = ep.tile([128, 64, D], F32, tag="e16c")
            for piece in range(8):
                nc.gpsimd.dma_gather(
                    out_ap=bw[:, piece * 8:(piece + 1) * 8, :],
                    in_ap=beta_e[:, :],
                    idxs_ap=bwin_sb[:, piece * 64:(piece + 1) * 64],
                    num_idxs=1024,
                    num_idxs_reg=1024,
                    elem_size=D,
                    single_packet=True,
                    queue_num=piece % 4,
                )
            bsel = ep.tile([128, 64, D], F32, tag="e16a")
            nc.vector.tensor_mul(out=bsel[:], in0=bw[:], in1=e8_sb[:])
            betad = ep.tile([128, 64], F32, tag="betad")
            nc.vector.reduce_sum(betad[:], bsel[:], axis=mybir.AxisListType.X)

            # ---- broadcast rank to all partitions as a free-dim row
            ident = ep.tile([128, 128], F32, tag="ident")
            make_identity(nc, ident[:])
            tps = psp.tile([64, 128], F32, tag="tp")
            nc.tensor.transpose(out=tps[:], in_=rank[:, :], identity=ident[:])
            tr = ep.tile([64, 128], F32, tag="tr")
            nc.vector.tensor_copy(out=tr[:], in_=tps[:])
            nc.sync.dma_start(
                out=rrow_d[0, :].rearrange("(a b) -> a b", b=128), in_=tr[:])
            rbc = ep.tile([128, NLAB], F32, tag="rbc")
            nc.sync.dma_start(out=rbc[:],
                              in_=rrow_d[0:1, :].to_broadcast([128, NLAB]))

            # ---- outer sums, rows sharded by osel
            osel_sb = ep.tile([128, 8, 64], F32, tag="osel")
            nc.sync.dma_start(out=osel_sb[:],
                              in_=osel_e[:, :].rearrange("p (t g) -> p t g", g=64))
            for t in range(4):
                for pn in range(2):
                    sel = wp.tile([128, 64], F32, tag="osel_t")
                    nc.vector.tensor_mul(out=sel[:], in0=betad[:],
                                         in1=osel_sb[:, pn * 4 + t, :])
                    bval = wp.tile([128, 1], F32, tag="bval")
                    nc.vector.reduce_sum(bval[:], sel[:], axis=mybir.AxisListType.X)
                    for half in range(2):
                        orow = op.tile([128, 2048], F32, tag="orow")
                        nc.vector.tensor_scalar_add(
                            out=orow[:],
                            in0=rbc[:, pn * 4096 + half * 2048:
                                    pn * 4096 + (half + 1) * 2048],
                            scalar1=bval[:, 0:1])
                        nc.sync.dma_start(
                            out=out_e[pn, t * 128:(t + 1) * 128,
                                      half * 2048:(half + 1) * 2048],
                            in_=orow[:])


# ---------------------------------------------------------------- entry
def kernel(emb_weight, beta_weight, alpha, edge_index, edge_label_index,
           num_users, num_items, scaling_factor):
    in_maps, meta = _prep(emb_weight, beta_weight, alpha, edge_index,
                          edge_label_index, num_users, scaling_factor)
    nc = _build(meta)
    nc.finalize()
    res = run_bass_kernel_spmd(nc, in_maps, list(range(NCORE)))
    pos = np.concatenate([res.results[k]["out"][0] for k in range(NCORE)], axis=0)
    neg = np.concatenate([res.results[k]["out"][1] for k in range(NCORE)], axis=0)
    return pos, neg


# revision 10
# speedup vs baseline: 1.0334x; 1.0334x over previous
"""LightGCN (AIM variant) distributed Bass kernel for 8 TRN2 NeuronCores.

Strategy (destination sharding):
  - 150000 nodes split into 8 slices of 18750 (padded to 18816 = 147*128 rows
    per slice; pad rows are always zero). Core k owns the destinations of
    slice k and all edges pointing into them (~500k edges/core).
  - z-substitution: with z = dis * x (dis = deg^-1/2), each LightGCN layer is
    x_{l+1}[c] = dis[c] * sum_{e in(c)} z_l[row_e]  -- a pure gather +
    segment-sum; the per-edge norm multiplier disappears.
  - z tables are bf16 [150528, 128]: each row's first 64 cols hold the
    embedding, the rest is never read (present only to satisfy the 256B
    dma_gather element granularity).
  - Per layer: each core gathers z rows for its edges (dma_gather custom
    instruction, 4 SWDGE queues, int16 indices -> the table is split into 5
    chunks of <=30112 rows), segment-sums them on the TensorEngine via
    on-device-built one-hot bf16 matrices (PSUM accumulation per
    128-destination group, 4 groups packed per PSUM bank), scales by dis,
    and AllGathers the new z slices.
  - Epilogue: item rows L2-normalized (*1.5), final out table AllGathered,
    the 2x8192 label endpoints gathered, ranks + beta terms computed on
    device, and the (4096, 4096) outer-sum outputs written (rows sharded
    across cores).
"""

import ml_dtypes
import numpy as np

import concourse.bass as bass
import concourse.bacc as bacc
import concourse.tile as tile
import concourse.mybir as mybir
from concourse.bass_utils import run_bass_kernel_spmd
from concourse.masks import make_identity

# ---------------------------------------------------------------- constants
N = 150000
D = 64
ZW = 128             # z-table row width (bf16) = 256B
NLAYERS = 3
NLAB = 8192
NCORE = 8
SLICE_REAL = 18750
SLICE_PAD = 18816    # 147 * 128
G_GROUPS = 147
SUPER = 4            # dest groups per super-group / PSUM bank
TABLE = NCORE * SLICE_PAD   # 150528
CHUNK = 30112
NCHUNK = 5
INSTR_MAX = 4096     # idxs per dma_gather instruction (single_packet=False)
IDXW = 4096          # idx window width in int16 columns (= 65536 idxs)
BETA_WIN = (N + 63) // 64   # 2344 windows of 64 floats
EPS = 1e-12

F32 = mybir.dt.float32
BF16 = mybir.dt.bfloat16
I16 = mybir.dt.int16


def _chunk_bounds():
    lo = [c * CHUNK for c in range(NCHUNK)]
    hi = [min((c + 1) * CHUNK, TABLE) for c in range(NCHUNK)]
    return lo, hi


def _zrow_locals():
    lo, hi = _chunk_bounds()
    out = []
    for c in range(NCHUNK):
        z = None
        for s in range(NCORE):
            zr = s * SLICE_PAD + SLICE_REAL
            if lo[c] <= zr < hi[c]:
                z = zr - lo[c]
                break
        assert z is not None
        out.append(z)
    return out


def _wrap16(flat):
    w = flat.reshape(-1, 16).T
    return np.tile(w, (8, 1)).copy()


# ---------------------------------------------------------------- host prep
def _prep(emb_weight, beta_weight, alpha, edge_index, edge_label_index,
          num_users, scaling_factor):
    row = np.asarray(edge_index[0]).astype(np.int64)
    col = np.asarray(edge_index[1]).astype(np.int64)
    num_users = int(num_users)

    deg = np.bincount(col, minlength=N).astype(np.int64)
    r_tab = (row // SLICE_REAL) * SLICE_PAD + (row % SLICE_REAL)
    core_of = col // SLICE_REAL
    zrl = _zrow_locals()

    NSG = (G_GROUPS + SUPER - 1) // SUPER     # 37 super-groups
    sg_of_g = np.arange(G_GROUPS) // SUPER

    # cell order: for sg, for chunk, for g in sg
    cell_rank = np.full((G_GROUPS, NCHUNK), -1, np.int64)
    cells = []               # rank -> (g, c)
    for sg in range(NSG):
        gs = range(sg * SUPER, min((sg + 1) * SUPER, G_GROUPS))
        for c in range(NCHUNK):
            for g in gs:
                cell_rank[g, c] = len(cells)
                cells.append((g, c))
    NCELL = len(cells)

    per_core = []
    counts = np.zeros((NCORE, NCELL), np.int64)
    for k in range(NCORE):
        m = core_of == k
        ld = col[m] - k * SLICE_REAL
        rt = r_tab[m]
        ch = rt // CHUNK
        g = ld // 128
        crk = cell_rank[g, ch]
        order = np.argsort(crk, kind="stable")
        ld, rt, crk = ld[order], rt[order], crk[order]
        counts[k] = np.bincount(crk, minlength=NCELL)
        per_core.append((ld, rt, crk))

    P = ((counts.max(axis=0) + 127) // 128) * 128        # [NCELL]
    cell_start = np.zeros(NCELL + 1, np.int64)
    cell_start[1:] = np.cumsum(P)
    S = int(cell_start[-1])
    SCOLS = S // 128
    assert P.max() <= INSTR_MAX, P.max()

    # per-128-block group id + first/last flags
    blk_g = np.empty(SCOLS, np.int64)
    for r, (g, c) in enumerate(cells):
        blk_g[cell_start[r] // 128: cell_start[r + 1] // 128] = g
    g_first_blk = np.full(G_GROUPS, -1, np.int64)
    g_last_blk = np.full(G_GROUPS, -1, np.int64)
    for b in range(SCOLS):
        g = blk_g[b]
        if g_first_blk[g] < 0:
            g_first_blk[g] = b
        g_last_blk[g] = b

    # instruction list: greedy fill from consecutive cells sharing (sg, chunk)
    instrs = []
    col_cur = 0
    r = 0
    while r < NCELL:
        g0, c0 = cells[r]
        sg0 = int(sg_of_g[g0])
        soff = int(cell_start[r])
        n = 0
        while r < NCELL and n + int(P[r]) <= INSTR_MAX:
            g, c = cells[r]
            if c != c0 or int(sg_of_g[g]) != sg0:
                break
            n += int(P[r])
            r += 1
        assert n > 0
        ncols = n // 16
        if col_cur % IDXW + ncols > IDXW:
            col_cur = (col_cur // IDXW + 1) * IDXW
        blocks = []
        for b in range(soff // 128, (soff + n) // 128):
            g = int(blk_g[b])
            blocks.append((b - soff // 128, g,
                           bool(b == g_first_blk[g]), bool(b == g_last_blk[g])))
        instrs.append(dict(c=c0, n=n, icol=col_cur, soff=soff, sg=sg0,
                           blocks=blocks))
        col_cur += ncols
    assert sum(i["n"] for i in instrs) == S
    TOTCOLS = ((col_cur + IDXW - 1) // IDXW) * IDXW

    slot2idx = np.empty(S, np.int64)
    for i in instrs:
        slot2idx[i["soff"]:i["soff"] + i["n"]] = np.arange(
            i["icol"] * 16, i["icol"] * 16 + i["n"])

    lab_src = np.asarray(edge_label_index[0]).astype(np.int64)
    lab_dst = np.asarray(edge_label_index[1]).astype(np.int64)
    lab_nodes = np.concatenate([lab_src, lab_dst])
    lab_tab = (lab_nodes // SLICE_REAL) * SLICE_PAD + (lab_nodes % SLICE_REAL)

    alpha = np.asarray(alpha, np.float32).reshape(-1)
    emb_weight = np.asarray(emb_weight, np.float32)
    beta_flat = np.zeros(BETA_WIN * 64, np.float32)
    beta_flat[:N] = np.asarray(beta_weight, np.float32).reshape(-1)

    chunk_of_cell = np.array([c for _, c in cells], np.int64)

    in_maps = []
    for k in range(NCORE):
        ld, rt, crk = per_core[k]
        nk = ld.shape[0]
        first_idx = np.zeros(NCELL, np.int64)
        cnt = counts[k]
        first_idx[1:] = np.cumsum(cnt)[:-1]
        pos_in_cell = np.arange(nk) - np.repeat(first_idx, cnt)
        slot = cell_start[crk] + pos_in_cell

        idx_flat = np.zeros(TOTCOLS * 16, np.int16)
        for i in instrs:
            idx_flat[i["icol"] * 16: i["icol"] * 16 + i["n"]] = zrl[i["c"]]
        idx_flat[slot2idx[slot]] = (rt - chunk_of_cell[crk] * CHUNK).astype(np.int16)

        dsel = np.full(S, -1.0, np.float32)
        dsel[slot] = (ld % 128).astype(np.float32)

        degs = np.zeros(SLICE_PAD, np.float32)
        degs[:SLICE_REAL] = deg[k * SLICE_REAL:(k + 1) * SLICE_REAL]
        mdeg = (degs > 0).astype(np.float32)
        ids = np.arange(k * SLICE_REAL, k * SLICE_REAL + SLICE_PAD)
        mitem = ((ids >= num_users) &
                 (ids < k * SLICE_REAL + SLICE_REAL)).astype(np.float32)

        embs = np.zeros((SLICE_PAD, D), np.float32)
        embs[:SLICE_REAL] = emb_weight[k * SLICE_REAL:(k + 1) * SLICE_REAL]

        lab_parts = []
        for c in range(NCHUNK):
            v = np.where(lab_tab // CHUNK == c, lab_tab - c * CHUNK,
                         zrl[c]).astype(np.int16)
            lab_parts.append(_wrap16(v))
        lab_idx = np.concatenate(lab_parts, axis=1)      # [128, 5*1024]

        bwin = _wrap16((lab_dst // 64).astype(np.int16))  # [128, 512]
        e8 = np.zeros((NLAB, 64), np.float32)
        e8[np.arange(NLAB), lab_dst % 64] = 1.0
        e8 = e8.reshape(64, 128, 64).transpose(1, 0, 2).copy()

        osel = np.zeros((8, 64), np.float32)
        for t in range(4):
            osel[t, k * 4 + t] = 1.0
            osel[4 + t, 32 + k * 4 + t] = 1.0
        osel = np.tile(osel[None, :, :], (128, 1, 1)).copy()

        in_maps.append({
            "emb": embs,
            "degf": degs.reshape(G_GROUPS, 128).T.copy(),
            "mdeg": mdeg.reshape(G_GROUPS, 128).T.copy(),
            "mitem": mitem.reshape(G_GROUPS, 128).T.copy(),
            "alpha": np.tile(alpha.reshape(1, 4), (128, 1)),
            "scal": np.full((128, 1), float(scaling_factor), np.float32),
            "idx": _wrap16(idx_flat),
            "dsel": dsel.reshape(-1, 128).T.astype(ml_dtypes.bfloat16),
            "lab": lab_idx,
            "bwin": bwin,
            "e8": e8.reshape(128, 64 * 64),
            "beta": beta_flat.reshape(BETA_WIN, 64),
            "iota": np.tile(np.arange(128, dtype=ml_dtypes.bfloat16)[None, :],
                            (128, 1)),
            "osel": osel.reshape(128, 8 * 64),
        })

    meta = dict(instrs=instrs, S=S, SCOLS=SCOLS, TOTCOLS=TOTCOLS, NSG=NSG)
    return in_maps, meta


# ---------------------------------------------------------------- builder
def _bc_mid(base_ap, p_count, mid_count, last_count):
    """[p, last] AP -> [p, mid(bcast), last]"""
    return bass.AP(base_ap.tensor, base_ap.offset,
                   [list(base_ap.ap[0])[:1] + [p_count],
                    [0, mid_count],
                    [1, last_count]])


def _bc_last(base_ap, p_count, mid_count, last_count):
    """[p, mid] AP -> [p, mid, last(bcast)]"""
    return bass.AP(base_ap.tensor, base_ap.offset,
                   [list(base_ap.ap[0])[:1] + [p_count],
                    [1, mid_count],
                    [0, last_count]])


def _build(meta):
    SCOLS = meta["SCOLS"]
    TOTCOLS = meta["TOTCOLS"]

    nc = bacc.Bacc(None, target_bir_lowering=False, num_swdge_queues=4)
    dp = nc.declare_dram_parameter
    emb_e = dp("emb", [SLICE_PAD, D], F32, isOutput=False)
    degf_e = dp("degf", [128, G_GROUPS], F32, isOutput=False)
    mdeg_e = dp("mdeg", [128, G_GROUPS], F32, isOutput=False)
    mitem_e = dp("mitem", [128, G_GROUPS], F32, isOutput=False)
    alpha_e = dp("alpha", [128, 4], F32, isOutput=False)
    scal_e = dp("scal", [128, 1], F32, isOutput=False)
    idx_e = dp("idx", [128, TOTCOLS], I16, isOutput=False)
    dsel_e = dp("dsel", [128, SCOLS], BF16, isOutput=False)
    lab_e = dp("lab", [128, NCHUNK * 1024], I16, isOutput=False)
    bwin_e = dp("bwin", [128, 512], I16, isOutput=False)
    e8_e = dp("e8", [128, 64 * 64], F32, isOutput=False)
    beta_e = dp("beta", [BETA_WIN, 64], F32, isOutput=False)
    iota_e = dp("iota", [128, 128], BF16, isOutput=False)
    osel_e = dp("osel", [128, 8 * 64], F32, isOutput=False)
    out_e = dp("out", [2, 512, 4096], F32, isOutput=True)

    zslice = [nc.dram_tensor(f"zs{l}", [SLICE_PAD, ZW], BF16)
              for l in range(NLAYERS)]
    zfull = [nc.dram_tensor(f"zf{l}", [TABLE, ZW], BF16, addr_space="Shared")
             for l in range(NLAYERS)]
    oslice = nc.dram_tensor("oslice", [SLICE_PAD, D], F32)
    ofull = nc.dram_tensor("ofull", [TABLE, D], F32, addr_space="Shared")
    rrow_d = nc.dram_tensor("rrow", [1, NLAB], F32)

    with tile.TileContext(nc) as tc:
        with (
            tc.tile_pool(name="persist", bufs=1) as pp,
            tc.tile_pool(name="psum", bufs=4, space="PSUM") as psp,
        ):
            out_acc = pp.tile([128, G_GROUPS, D], F32)
            dis = pp.tile([128, G_GROUPS], F32)
            alpha_sb = pp.tile([128, 4], F32)
            scal_sb = pp.tile([128, 1], F32)
            mitem_sb = pp.tile([128, G_GROUPS], F32)

            nc.sync.dma_start(out=alpha_sb[:], in_=alpha_e[:, :])
            nc.sync.dma_start(out=scal_sb[:], in_=scal_e[:, :])
            nc.sync.dma_start(out=mitem_sb[:], in_=mitem_e[:, :])
            _layers(nc, tc, meta, locals())
            _epilogue(nc, tc, meta, locals())
    return nc


def _layers(nc, tc, meta, env):
    instrs = meta["instrs"]
    SCOLS = meta["SCOLS"]
    clo, chi = _chunk_bounds()
    mul = mybir.AluOpType.mult
    out_acc = env["out_acc"]; dis = env["dis"]; alpha_sb = env["alpha_sb"]
    psp = env["psp"]
    emb_e = env["emb_e"]; degf_e = env["degf_e"]; mdeg_e = env["mdeg_e"]
    dsel_e = env["dsel_e"]; iota_e = env["iota_e"]; idx_e = env["idx_e"]
    zslice = env["zslice"]; zfull = env["zfull"]
    rg = [list(range(NCORE))]
    with (
        tc.tile_pool(name="work", bufs=3) as wp,
        tc.tile_pool(name="gat", bufs=6) as gp,
        tc.tile_pool(name="bmat", bufs=8) as bp,
        tc.tile_pool(name="idxw", bufs=2) as ip,
        tc.tile_pool(name="lpersist", bufs=1) as lp,
    ):
            dsel_sb = lp.tile([128, SCOLS], BF16)
            iota_sb = lp.tile([128, 128], BF16)
            nc.sync.dma_start(out=dsel_sb[:], in_=dsel_e[:, :])
            nc.sync.dma_start(out=iota_sb[:], in_=iota_e[:, :])

            # ---- dis = (deg > 0) / sqrt(max(deg, 1))
            degf = wp.tile([128, G_GROUPS], F32, tag="deg")
            mdeg = wp.tile([128, G_GROUPS], F32, tag="deg")
            nc.sync.dma_start(out=degf[:], in_=degf_e[:, :])
            nc.sync.dma_start(out=mdeg[:], in_=mdeg_e[:, :])
            degc = wp.tile([128, G_GROUPS], F32, tag="deg")
            nc.vector.tensor_scalar_max(out=degc[:], in0=degf[:], scalar1=1.0)
            dsq = wp.tile([128, G_GROUPS], F32, tag="deg")
            nc.scalar.activation(out=dsq[:], in_=degc[:],
                                 func=mybir.ActivationFunctionType.Sqrt)
            drc = wp.tile([128, G_GROUPS], F32, tag="deg")
            nc.vector.reciprocal(out=drc[:], in_=dsq[:])
            nc.vector.tensor_mul(out=dis[:], in0=drc[:], in1=mdeg[:])
            dis2 = lp.tile([128, G_GROUPS], F32)
            nc.vector.tensor_mul(out=dis2[:], in0=dis[:], in1=dis[:])

            # ---- z0 slice + out_acc init (hi/lo bf16 split of z)
            NSGL = (G_GROUPS + SUPER - 1) // SUPER
            for sg in range(NSGL):
                g0 = sg * SUPER
                W = min(SUPER, G_GROUPS - g0)
                et = wp.tile([128, SUPER, D], F32, tag="emb")
                nc.sync.dma_start(
                    out=et[:, :W, :],
                    in_=emb_e[g0 * 128:(g0 + W) * 128, :].rearrange(
                        "(g p) w -> p g w", p=128))
                z032 = wp.tile([128, SUPER, D], F32, tag="z032")
                nc.vector.tensor_tensor(out=z032[:, :W, :], in0=et[:, :W, :],
                                        in1=_bc_last(dis[:, g0:g0 + W], 128, W, D),
                                        op=mul)
                zh = wp.tile([128, SUPER, D], BF16, tag="zh")
                nc.vector.tensor_copy(out=zh[:, :W, :], in_=z032[:, :W, :])
                hi32 = wp.tile([128, SUPER, D], F32, tag="hi32")
                nc.vector.tensor_copy(out=hi32[:, :W, :], in_=zh[:, :W, :])
                zl = wp.tile([128, SUPER, D], BF16, tag="zl")
                nc.vector.tensor_tensor(out=zl[:, :W, :], in0=z032[:, :W, :],
                                        in1=hi32[:, :W, :],
                                        op=mybir.AluOpType.subtract)
                nc.sync.dma_start(
                    out=zslice[0][g0 * 128:(g0 + W) * 128, 0:D].rearrange(
                        "(g p) w -> p g w", p=128),
                    in_=zh[:, :W, :])
                nc.sync.dma_start(
                    out=zslice[0][g0 * 128:(g0 + W) * 128, D:ZW].rearrange(
                        "(g p) w -> p g w", p=128),
                    in_=zl[:, :W, :])
                nc.vector.tensor_tensor(
                    out=out_acc[:, g0:g0 + W, :], in0=et[:, :W, :],
                    in1=alpha_sb[:, 0:1].to_broadcast([128, W, D]),
                    op=mul)
            nc.gpsimd.collective_compute(
                "AllGather", mybir.AluOpType.bypass, replica_groups=rg,
                ins=[zslice[0].ap().opt()], outs=[zfull[0].ap().opt()])

            # ---- propagation layers
            qrr = 0
            disal = lp.tile([128, NLAYERS, G_GROUPS], F32)
            for l in range(NLAYERS):
                nc.vector.tensor_scalar(out=disal[:, l, :], in0=dis[:],
                                        scalar1=alpha_sb[:, l + 1:l + 2],
                                        scalar2=None, op0=mul)
            for l in range(NLAYERS):
                zf = zfull[l]
                win_tile = None
                win_id = -1
                state = dict(sg=-1, psum=None, done=[], started=False)

                def flush_groups():
                    psum = state["psum"]
                    for g in state["done"]:
                        gl = g % SUPER
                        xs = wp.tile([128, D], F32, tag="xs")
                        ps_hl = bass.AP(psum.tensor, psum.offset + gl * ZW,
                                        [list(psum.ap[0])[:1] + [128],
                                         [1, D], [D, 2]])
                        nc.vector.reduce_sum(xs[:], ps_hl,
                                             axis=mybir.AxisListType.X)
                        if l < NLAYERS - 1:
                            zn32 = wp.tile([128, D], F32, tag="zn32")
                            nc.vector.tensor_scalar(
                                out=zn32[:], in0=xs[:],
                                scalar1=dis[:, g:g + 1],
                                scalar2=dis[:, g:g + 1], op0=mul, op1=mul)
                            zh = wp.tile([128, D], BF16, tag="zh")
                            nc.vector.tensor_copy(out=zh[:], in_=zn32[:])
                            hi32 = wp.tile([128, D], F32, tag="hi32")
                            nc.vector.tensor_copy(out=hi32[:], in_=zh[:])
                            zl = wp.tile([128, D], BF16, tag="zl")
                            nc.vector.tensor_tensor(
                                out=zl[:], in0=zn32[:], in1=hi32[:],
                                op=mybir.AluOpType.subtract)
                            nc.sync.dma_start(
                                out=zslice[l + 1][g * 128:(g + 1) * 128, 0:D],
                                in_=zh[:])
                            nc.sync.dma_start(
                                out=zslice[l + 1][g * 128:(g + 1) * 128, D:ZW],
                                in_=zl[:])
                        t2 = wp.tile([128, D], F32, tag="t2")
                        nc.vector.tensor_scalar(
                            out=t2[:], in0=xs[:],
                            scalar1=dis[:, g:g + 1],
                            scalar2=alpha_sb[:, l + 1:l + 2], op0=mul, op1=mul)
                        nc.vector.tensor_add(out=out_acc[:, g, :],
                                             in0=out_acc[:, g, :], in1=t2[:])
                    state["done"] = []

                for ins in instrs:
                    c, n, icol, soff, sg = (ins["c"], ins["n"], ins["icol"],
                                            ins["soff"], ins["sg"])
                    if sg != state["sg"]:
                        flush_groups()
                        state["psum"] = psp.tile([128, SUPER * ZW], F32, tag="acc", name="psacc")
                        state["sg"] = sg
                        state["started"] = False
                    psum = state["psum"]
                    nb = n // 128
                    w = icol // IDXW
                    if w != win_id:
                        win_tile = ip.tile([128, IDXW], I16, tag="idxw")
                        nc.sync.dma_start(
                            out=win_tile[:],
                            in_=idx_e[:, w * IDXW:(w + 1) * IDXW])
                        win_id = w
                    gt = gp.tile([128, INSTR_MAX // 128, ZW], BF16, tag="gat")
                    wc = icol % IDXW
                    nc.gpsimd.dma_gather(
                        out_ap=gt[:, :nb, :],
                        in_ap=zf[clo[c]:chi[c], :],
                        idxs_ap=win_tile[:, wc:wc + n // 16],
                        num_idxs=n,
                        num_idxs_reg=n,
                        elem_size=ZW,
                        single_packet=False,
                        queue_num=qrr % 4,
                    )
                    qrr += 1
                    col0 = soff // 128
                    for p0 in range(0, nb, 8):
                        pnb = min(8, nb - p0)
                        bt = bp.tile([128, 8, 128], BF16, tag="bmat")
                        in0 = dsel_sb[:, col0 + p0:col0 + p0 + pnb
                                      ].to_broadcast([128, pnb, 128])
                        in1 = _bc_mid(iota_sb[:], 128, pnb, 128)
                        nc.vector.tensor_tensor(out=bt[:, :pnb, :], in0=in0,
                                                in1=in1,
                                                op=mybir.AluOpType.is_equal)
                        for j in range(pnb):
                            jj = p0 + j
                            _, g, first, last = ins["blocks"][jj]
                            gl = g % SUPER
                            nc.tensor.matmul(
                                psum[:, gl * ZW:(gl + 1) * ZW],
                                bt[:, j, :], gt[:, jj, :],
                                start=not state["started"], stop=last,
                                skip_group_check=True)
                            state["started"] = True
                            if last:
                                state["done"].append(g)
                flush_groups()
                if l < NLAYERS - 1:
                    nc.gpsimd.collective_compute(
                        "AllGather", mybir.AluOpType.bypass, replica_groups=rg,
                        ins=[zslice[l + 1].ap().opt()],
                        outs=[zfull[l + 1].ap().opt()])


def _epilogue(nc, tc, meta, env):
    clo, chi = _chunk_bounds()
    mul = mybir.AluOpType.mult
    add = mybir.AluOpType.add
    out_acc = env["out_acc"]; dis = env["dis"]; scal_sb = env["scal_sb"]
    mitem_sb = env["mitem_sb"]; psp = env["psp"]
    oslice = env["oslice"]; ofull = env["ofull"]; rrow_d = env["rrow_d"]
    lab_e = env["lab_e"]; bwin_e = env["bwin_e"]; e8_e = env["e8_e"]
    beta_e = env["beta_e"]; osel_e = env["osel_e"]; out_e = env["out_e"]
    pp = env["pp"]
    rg = [list(range(NCORE))]
    with (
        tc.tile_pool(name="ep", bufs=1) as ep,
        tc.tile_pool(name="ew", bufs=3) as wp,
        tc.tile_pool(name="eg", bufs=2) as gp,
        tc.tile_pool(name="orow", bufs=2) as op,
    ):
            # ---- item normalization + out slice
            for g in range(G_GROUPS):
                sq = wp.tile([128, D], F32, tag="sq")
                nc.vector.tensor_mul(out=sq[:], in0=out_acc[:, g, :],
                                     in1=out_acc[:, g, :])
                ss = wp.tile([128, 1], F32, tag="ss")
                nc.vector.reduce_sum(ss[:], sq[:], axis=mybir.AxisListType.X)
                nrm = wp.tile([128, 1], F32, tag="ss")
                nc.scalar.activation(out=nrm[:], in_=ss[:],
                                     func=mybir.ActivationFunctionType.Sqrt)
                nc.vector.tensor_scalar_max(out=nrm[:], in0=nrm[:], scalar1=EPS)
                rec = wp.tile([128, 1], F32, tag="ss")
                nc.vector.reciprocal(out=rec[:], in_=nrm[:])
                fac = wp.tile([128, 1], F32, tag="ss")
                nc.vector.tensor_scalar(out=fac[:], in0=rec[:],
                                        scalar1=scal_sb[:, 0:1], scalar2=-1.0,
                                        op0=mul, op1=add)
                nc.vector.tensor_scalar(out=fac[:], in0=fac[:],
                                        scalar1=mitem_sb[:, g:g + 1], scalar2=1.0,
                                        op0=mul, op1=add)
                on = wp.tile([128, D], F32, tag="on")
                nc.vector.tensor_scalar(out=on[:], in0=out_acc[:, g, :],
                                        scalar1=fac[:, 0:1], scalar2=None,
                                        op0=mul)
                nc.sync.dma_start(out=oslice[g * 128:(g + 1) * 128, :], in_=on[:])
            nc.gpsimd.collective_compute(
                "AllGather", mybir.AluOpType.bypass, replica_groups=rg,
                ins=[oslice.ap().opt()], outs=[ofull.ap().opt()])

            # ---- label gathers (16384 slots, 5 chunk passes summed)
            lab_sb = ep.tile([128, NCHUNK * 1024], I16, tag="lab")
            nc.sync.dma_start(out=lab_sb[:], in_=lab_e[:, :])
            acc_lab = pp.tile([128, 128, D], F32, tag="out_acc")
            for c in range(NCHUNK):
                for half in range(2):
                    lt = gp.tile([128, 64, D], F32, tag="labg")
                    for piece in range(8):
                        pc = half * 8 + piece
                        nc.gpsimd.dma_gather(
                            out_ap=lt[:, piece * 8:(piece + 1) * 8, :],
                            in_ap=ofull[clo[c]:chi[c], :],
                            idxs_ap=lab_sb[:, c * 1024 + pc * 64:
                                           c * 1024 + (pc + 1) * 64],
                            num_idxs=1024,
                            num_idxs_reg=1024,
                            elem_size=D,
                            single_packet=True,
                            queue_num=pc % 4,
                        )
                    dst = acc_lab[:, half * 64:(half + 1) * 64, :]
                    if c == 0:
                        nc.vector.tensor_copy(out=dst, in_=lt[:])
                    else:
                        nc.vector.tensor_add(out=dst, in0=dst, in1=lt[:])

            # rank[i] at [i%128, i//128]; src slots g 0..63, dst slots g 64..127
            rmul = ep.tile([128, 64, D], F32, tag="e16a")
            nc.vector.tensor_mul(out=rmul[:], in0=acc_lab[:, :64, :],
                                 in1=acc_lab[:, 64:, :])
            rank = ep.tile([128, 64], F32, tag="rank")
            nc.vector.reduce_sum(rank[:], rmul[:], axis=mybir.AxisListType.X)

            # ---- beta windows
            bwin_sb = ep.tile([128, 512], I16, tag="bwin")
            nc.sync.dma_start(out=bwin_sb[:], in_=bwin_e[:, :])
            e8_sb = ep.tile([128, 64, D], F32, tag="e16b")
            nc.sync.dma_start(out=e8_sb[:],
                              in_=e8_e[:, :].rearrange("p (g d) -> p g d", d=D))
            bw = ep.tile([128, 64, D], F32, tag="e16c")
            for piece in range(8):
                nc.gpsimd.dma_gather(
                    out_ap=bw[:, piece * 8:(piece + 1) * 8, :],
                    in_ap=beta_e[:, :],
                    idxs_ap=bwin_sb[:, piece * 64:(piece + 1) * 64],
                    num_idxs=1024,
                    num_idxs_reg=1024,
                    elem_size=D,
                    single_packet=True,
                    queue_num=piece % 4,
                )
            bsel = ep.tile([128, 64, D], F32, tag="e16a")
            nc.vector.tensor_mul(out=bsel[:], in0=bw[:], in1=e8_sb[:])
            betad = ep.tile([128, 64], F32, tag="betad")
            nc.vector.reduce_sum(betad[:], bsel[:], axis=mybir.AxisListType.X)

            # ---- broadcast rank to all partitions as a free-dim row
            ident = ep.tile([128, 128], F32, tag="ident")
            make_identity(nc, ident[:])
            tps = psp.tile([64, 128], F32, tag="tp")
            nc.tensor.transpose(out=tps[:], in_=rank[:, :], identity=ident[:])
            tr = ep.tile([64, 128], F32, tag="tr")
            nc.vector.tensor_copy(out=tr[:], in_=tps[:])
            nc.sync.dma_start(
                out=rrow_d[0, :].rearrange("(a b) -> a b", b=128), in_=tr[:])
            rbc = ep.tile([128, NLAB], F32, tag="rbc")
            nc.sync.dma_start(out=rbc[:],
                              in_=rrow_d[0:1, :].to_broadcast([128, NLAB]))

            # ---- outer sums, rows sharded by osel
            osel_sb = ep.tile([128, 8, 64], F32, tag="osel")
            nc.sync.dma_start(out=osel_sb[:],
                              in_=osel_e[:, :].rearrange("p (t g) -> p t g", g=64))
            for t in range(4):
                for pn in range(2):
                    sel = wp.tile([128, 64], F32, tag="osel_t")
                    nc.vector.tensor_mul(out=sel[:], in0=betad[:],
                                         in1=osel_sb[:, pn * 4 + t, :])
                    bval = wp.tile([128, 1], F32, tag="bval")
                    nc.vector.reduce_sum(bval[:], sel[:], axis=mybir.AxisListType.X)
                    for half in range(2):
                        orow = op.tile([128, 2048], F32, tag="orow")
                        nc.vector.tensor_scalar_add(
                            out=orow[:],
                            in0=rbc[:, pn * 4096 + half * 2048:
                                    pn * 4096 + (half + 1) * 2048],
                            scalar1=bval[:, 0:1])
                        nc.sync.dma_start(
                            out=out_e[pn, t * 128:(t + 1) * 128,
                                      half * 2048:(half + 1) * 2048],
                            in_=orow[:])


# ---------------------------------------------------------------- entry
def kernel(emb_weight, beta_weight, alpha, edge_index, edge_label_index,
           num_users, num_items, scaling_factor):
    in_maps, meta = _prep(emb_weight, beta_weight, alpha, edge_index,
                          edge_label_index, num_users, scaling_factor)
    nc = _build(meta)
    nc.finalize()
    res = run_bass_kernel_spmd(nc, in_maps, list(range(NCORE)))
    pos = np.concatenate([res.results[k]["out"][0] for k in range(NCORE)], axis=0)
    neg = np.concatenate([res.results[k]["out"][1] for k in range(NCORE)], axis=0)
    return pos, neg


# revision 11
# speedup vs baseline: 1.2144x; 1.1752x over previous
"""LightGCN (AIM variant) distributed Bass kernel for 8 TRN2 NeuronCores.

Strategy (destination sharding):
  - 150000 nodes split into 8 slices of 18750 (padded to 18816 = 147*128 rows
    per slice; pad rows are always zero). Core k owns the destinations of
    slice k and all edges pointing into them (~500k edges/core).
  - z-substitution: with z = dis * x (dis = deg^-1/2), each LightGCN layer is
    x_{l+1}[c] = dis[c] * sum_{e in(c)} z_l[row_e]  -- a pure gather +
    segment-sum; the per-edge norm multiplier disappears.
  - z tables are bf16 [150528, 128]: each row's first 64 cols hold the
    embedding, the rest is never read (present only to satisfy the 256B
    dma_gather element granularity).
  - Per layer: each core gathers z rows for its edges (dma_gather custom
    instruction, 4 SWDGE queues, int16 indices -> the table is split into 5
    chunks of <=30112 rows), segment-sums them on the TensorEngine via
    on-device-built one-hot bf16 matrices (PSUM accumulation per
    128-destination group, 4 groups packed per PSUM bank), scales by dis,
    and AllGathers the new z slices.
  - Epilogue: item rows L2-normalized (*1.5), final out table AllGathered,
    the 2x8192 label endpoints gathered, ranks + beta terms computed on
    device, and the (4096, 4096) outer-sum outputs written (rows sharded
    across cores).
"""

import ml_dtypes
import numpy as np

import concourse.bass as bass
import concourse.bacc as bacc
import concourse.tile as tile
import concourse.mybir as mybir
from concourse.bass_utils import run_bass_kernel_spmd
from concourse.masks import make_identity

# ---------------------------------------------------------------- constants
N = 150000
D = 64
ZW = 128             # z-table row width (bf16) = 256B
NLAYERS = 3
NLAB = 8192
NCORE = 8
SLICE_REAL = 18750
SLICE_PAD = 18816    # 147 * 128
G_GROUPS = 147
SUPER = 4            # dest groups per super-group / PSUM bank
TABLE = NCORE * SLICE_PAD   # 150528
CHUNK = 30112
NCHUNK = 5
INSTR_MAX = 4096     # idxs per dma_gather instruction (single_packet=False)
IDXW = 4096          # idx window width in int16 columns (= 65536 idxs)
BETA_WIN = (N + 63) // 64   # 2344 windows of 64 floats
EPS = 1e-12

F32 = mybir.dt.float32
BF16 = mybir.dt.bfloat16
I16 = mybir.dt.int16


def _chunk_bounds():
    lo = [c * CHUNK for c in range(NCHUNK)]
    hi = [min((c + 1) * CHUNK, TABLE) for c in range(NCHUNK)]
    return lo, hi


def _zrow_locals():
    lo, hi = _chunk_bounds()
    out = []
    for c in range(NCHUNK):
        z = None
        for s in range(NCORE):
            zr = s * SLICE_PAD + SLICE_REAL
            if lo[c] <= zr < hi[c]:
                z = zr - lo[c]
                break
        assert z is not None
        out.append(z)
    return out


def _wrap16(flat):
    w = flat.reshape(-1, 16).T
    return np.tile(w, (8, 1)).copy()


# ---------------------------------------------------------------- host prep
def _prep(emb_weight, beta_weight, alpha, edge_index, edge_label_index,
          num_users, scaling_factor):
    row = np.asarray(edge_index[0]).astype(np.int64)
    col = np.asarray(edge_index[1]).astype(np.int64)
    num_users = int(num_users)

    deg = np.bincount(col, minlength=N).astype(np.int64)
    r_tab = (row // SLICE_REAL) * SLICE_PAD + (row % SLICE_REAL)
    core_of = col // SLICE_REAL
    zrl = _zrow_locals()

    NSG = (G_GROUPS + SUPER - 1) // SUPER     # 37 super-groups
    sg_of_g = np.arange(G_GROUPS) // SUPER

    # cell order: for sg, for chunk, for g in sg
    cell_rank = np.full((G_GROUPS, NCHUNK), -1, np.int64)
    cells = []               # rank -> (g, c)
    for sg in range(NSG):
        gs = range(sg * SUPER, min((sg + 1) * SUPER, G_GROUPS))
        for c in range(NCHUNK):
            for g in gs:
                cell_rank[g, c] = len(cells)
                cells.append((g, c))
    NCELL = len(cells)

    per_core = []
    counts = np.zeros((NCORE, NCELL), np.int64)
    for k in range(NCORE):
        m = core_of == k
        ld = col[m] - k * SLICE_REAL
        rt = r_tab[m]
        ch = rt // CHUNK
        g = ld // 128
        crk = cell_rank[g, ch]
        order = np.argsort(crk, kind="stable")
        ld, rt, crk = ld[order], rt[order], crk[order]
        counts[k] = np.bincount(crk, minlength=NCELL)
        per_core.append((ld, rt, crk))

    P = ((counts.max(axis=0) + 127) // 128) * 128        # [NCELL]
    cell_start = np.zeros(NCELL + 1, np.int64)
    cell_start[1:] = np.cumsum(P)
    S = int(cell_start[-1])
    SCOLS = S // 128
    assert P.max() <= INSTR_MAX, P.max()

    # per-128-block group id + first/last flags
    blk_g = np.empty(SCOLS, np.int64)
    for r, (g, c) in enumerate(cells):
        blk_g[cell_start[r] // 128: cell_start[r + 1] // 128] = g
    g_first_blk = np.full(G_GROUPS, -1, np.int64)
    g_last_blk = np.full(G_GROUPS, -1, np.int64)
    for b in range(SCOLS):
        g = blk_g[b]
        if g_first_blk[g] < 0:
            g_first_blk[g] = b
        g_last_blk[g] = b

    # instruction list: greedy fill from consecutive cells sharing (sg, chunk)
    instrs = []
    col_cur = 0
    r = 0
    while r < NCELL:
        g0, c0 = cells[r]
        sg0 = int(sg_of_g[g0])
        soff = int(cell_start[r])
        n = 0
        while r < NCELL and n + int(P[r]) <= INSTR_MAX:
            g, c = cells[r]
            if c != c0 or int(sg_of_g[g]) != sg0:
                break
            n += int(P[r])
            r += 1
        assert n > 0
        ncols = n // 16
        if col_cur % IDXW + ncols > IDXW:
            col_cur = (col_cur // IDXW + 1) * IDXW
        blocks = []
        for b in range(soff // 128, (soff + n) // 128):
            g = int(blk_g[b])
            blocks.append((b - soff // 128, g,
                           bool(b == g_first_blk[g]), bool(b == g_last_blk[g])))
        instrs.append(dict(c=c0, n=n, icol=col_cur, soff=soff, sg=sg0,
                           blocks=blocks))
        col_cur += ncols
    assert sum(i["n"] for i in instrs) == S
    TOTCOLS = ((col_cur + IDXW - 1) // IDXW) * IDXW

    slot2idx = np.empty(S, np.int64)
    for i in instrs:
        slot2idx[i["soff"]:i["soff"] + i["n"]] = np.arange(
            i["icol"] * 16, i["icol"] * 16 + i["n"])

    lab_src = np.asarray(edge_label_index[0]).astype(np.int64)
    lab_dst = np.asarray(edge_label_index[1]).astype(np.int64)
    lab_nodes = np.concatenate([lab_src, lab_dst])
    lab_tab = (lab_nodes // SLICE_REAL) * SLICE_PAD + (lab_nodes % SLICE_REAL)

    alpha = np.asarray(alpha, np.float32).reshape(-1)
    emb_weight = np.asarray(emb_weight, np.float32)
    beta_flat = np.zeros(BETA_WIN * 64, np.float32)
    beta_flat[:N] = np.asarray(beta_weight, np.float32).reshape(-1)

    chunk_of_cell = np.array([c for _, c in cells], np.int64)

    in_maps = []
    for k in range(NCORE):
        ld, rt, crk = per_core[k]
        nk = ld.shape[0]
        first_idx = np.zeros(NCELL, np.int64)
        cnt = counts[k]
        first_idx[1:] = np.cumsum(cnt)[:-1]
        pos_in_cell = np.arange(nk) - np.repeat(first_idx, cnt)
        slot = cell_start[crk] + pos_in_cell

        idx_flat = np.zeros(TOTCOLS * 16, np.int16)
        for i in instrs:
            idx_flat[i["icol"] * 16: i["icol"] * 16 + i["n"]] = zrl[i["c"]]
        idx_flat[slot2idx[slot]] = (rt - chunk_of_cell[crk] * CHUNK).astype(np.int16)

        dsel = np.full(S, -1.0, np.float32)
        dsel[slot] = (ld % 128).astype(np.float32)

        degs = np.zeros(SLICE_PAD, np.float32)
        degs[:SLICE_REAL] = deg[k * SLICE_REAL:(k + 1) * SLICE_REAL]
        mdeg = (degs > 0).astype(np.float32)
        ids = np.arange(k * SLICE_REAL, k * SLICE_REAL + SLICE_PAD)
        mitem = ((ids >= num_users) &
                 (ids < k * SLICE_REAL + SLICE_REAL)).astype(np.float32)

        embs = np.zeros((SLICE_PAD, D), np.float32)
        embs[:SLICE_REAL] = emb_weight[k * SLICE_REAL:(k + 1) * SLICE_REAL]

        lab_parts = []
        for c in range(NCHUNK):
            v = np.where(lab_tab // CHUNK == c, lab_tab - c * CHUNK,
                         zrl[c]).astype(np.int16)
            lab_parts.append(_wrap16(v))
        lab_idx = np.concatenate(lab_parts, axis=1)      # [128, 5*1024]

        bwin = _wrap16((lab_dst // 64).astype(np.int16))  # [128, 512]
        e8 = np.zeros((NLAB, 64), np.float32)
        e8[np.arange(NLAB), lab_dst % 64] = 1.0
        e8 = e8.reshape(64, 128, 64).transpose(1, 0, 2).copy()

        osel = np.zeros((8, 64), np.float32)
        for t in range(4):
            osel[t, k * 4 + t] = 1.0
            osel[4 + t, 32 + k * 4 + t] = 1.0
        osel = np.tile(osel[None, :, :], (128, 1, 1)).copy()

        in_maps.append({
            "emb": embs,
            "degf": degs.reshape(G_GROUPS, 128).T.copy(),
            "mdeg": mdeg.reshape(G_GROUPS, 128).T.copy(),
            "mitem": mitem.reshape(G_GROUPS, 128).T.copy(),
            "alpha": np.tile(alpha.reshape(1, 4), (128, 1)),
            "scal": np.full((128, 1), float(scaling_factor), np.float32),
            "idx": _wrap16(idx_flat),
            "dsel": dsel.reshape(-1, 128).T.astype(ml_dtypes.bfloat16),
            "lab": lab_idx,
            "bwin": bwin,
            "e8": e8.reshape(128, 64 * 64),
            "beta": beta_flat.reshape(BETA_WIN, 64),
            "iota": np.tile(np.arange(128, dtype=ml_dtypes.bfloat16)[None, :],
                            (128, 1)),
            "osel": osel.reshape(128, 8 * 64),
        })

    meta = dict(instrs=instrs, S=S, SCOLS=SCOLS, TOTCOLS=TOTCOLS, NSG=NSG)
    return in_maps, meta


# ---------------------------------------------------------------- builder
def _bc_mid(base_ap, p_count, mid_count, last_count):
    """[p, last] AP -> [p, mid(bcast), last]"""
    return bass.AP(base_ap.tensor, base_ap.offset,
                   [list(base_ap.ap[0])[:1] + [p_count],
                    [0, mid_count],
                    [1, last_count]])


def _bc_last(base_ap, p_count, mid_count, last_count):
    """[p, mid] AP -> [p, mid, last(bcast)]"""
    return bass.AP(base_ap.tensor, base_ap.offset,
                   [list(base_ap.ap[0])[:1] + [p_count],
                    [1, mid_count],
                    [0, last_count]])


def _build(meta):
    SCOLS = meta["SCOLS"]
    TOTCOLS = meta["TOTCOLS"]

    nc = bacc.Bacc(None, target_bir_lowering=False, num_swdge_queues=4)
    dp = nc.declare_dram_parameter
    emb_e = dp("emb", [SLICE_PAD, D], F32, isOutput=False)
    degf_e = dp("degf", [128, G_GROUPS], F32, isOutput=False)
    mdeg_e = dp("mdeg", [128, G_GROUPS], F32, isOutput=False)
    mitem_e = dp("mitem", [128, G_GROUPS], F32, isOutput=False)
    alpha_e = dp("alpha", [128, 4], F32, isOutput=False)
    scal_e = dp("scal", [128, 1], F32, isOutput=False)
    idx_e = dp("idx", [128, TOTCOLS], I16, isOutput=False)
    dsel_e = dp("dsel", [128, SCOLS], BF16, isOutput=False)
    lab_e = dp("lab", [128, NCHUNK * 1024], I16, isOutput=False)
    bwin_e = dp("bwin", [128, 512], I16, isOutput=False)
    e8_e = dp("e8", [128, 64 * 64], F32, isOutput=False)
    beta_e = dp("beta", [BETA_WIN, 64], F32, isOutput=False)
    iota_e = dp("iota", [128, 128], BF16, isOutput=False)
    osel_e = dp("osel", [128, 8 * 64], F32, isOutput=False)
    out_e = dp("out", [2, 512, 4096], F32, isOutput=True)

    zslice = [nc.dram_tensor(f"zs{l}", [SLICE_PAD, ZW], BF16)
              for l in range(NLAYERS)]
    zfull = [nc.dram_tensor(f"zf{l}", [TABLE, ZW], BF16, addr_space="Shared")
             for l in range(NLAYERS)]
    oslice = nc.dram_tensor("oslice", [SLICE_PAD, D], F32)
    ofull = nc.dram_tensor("ofull", [TABLE, D], F32, addr_space="Shared")
    rrow_d = nc.dram_tensor("rrow", [1, NLAB], F32)

    with tile.TileContext(nc) as tc:
        with (
            tc.tile_pool(name="persist", bufs=1) as pp,
            tc.tile_pool(name="psum", bufs=4, space="PSUM") as psp,
        ):
            out_acc = pp.tile([128, G_GROUPS, D], F32)
            dis = pp.tile([128, G_GROUPS], F32)
            alpha_sb = pp.tile([128, 4], F32)
            scal_sb = pp.tile([128, 1], F32)
            mitem_sb = pp.tile([128, G_GROUPS], F32)

            nc.sync.dma_start(out=alpha_sb[:], in_=alpha_e[:, :])
            nc.sync.dma_start(out=scal_sb[:], in_=scal_e[:, :])
            nc.sync.dma_start(out=mitem_sb[:], in_=mitem_e[:, :])
            _layers(nc, tc, meta, locals())
            _epilogue(nc, tc, meta, locals())
    return nc


def _layers(nc, tc, meta, env):
    instrs = meta["instrs"]
    SCOLS = meta["SCOLS"]
    clo, chi = _chunk_bounds()
    mul = mybir.AluOpType.mult
    out_acc = env["out_acc"]; dis = env["dis"]; alpha_sb = env["alpha_sb"]
    psp = env["psp"]
    emb_e = env["emb_e"]; degf_e = env["degf_e"]; mdeg_e = env["mdeg_e"]
    dsel_e = env["dsel_e"]; iota_e = env["iota_e"]; idx_e = env["idx_e"]
    zslice = env["zslice"]; zfull = env["zfull"]
    rg = [list(range(NCORE))]
    with (
        tc.tile_pool(name="work", bufs=3) as wp,
        tc.tile_pool(name="gat", bufs=6) as gp,
        tc.tile_pool(name="bmat", bufs=8) as bp,
        tc.tile_pool(name="idxw", bufs=2) as ip,
        tc.tile_pool(name="lpersist", bufs=1) as lp,
    ):
            dsel_sb = lp.tile([128, SCOLS], BF16)
            iota_sb = lp.tile([128, 128], BF16)
            nc.sync.dma_start(out=dsel_sb[:], in_=dsel_e[:, :])
            nc.sync.dma_start(out=iota_sb[:], in_=iota_e[:, :])

            # ---- dis = (deg > 0) / sqrt(max(deg, 1))
            degf = wp.tile([128, G_GROUPS], F32, tag="deg")
            mdeg = wp.tile([128, G_GROUPS], F32, tag="deg")
            nc.sync.dma_start(out=degf[:], in_=degf_e[:, :])
            nc.sync.dma_start(out=mdeg[:], in_=mdeg_e[:, :])
            degc = wp.tile([128, G_GROUPS], F32, tag="deg")
            nc.vector.tensor_scalar_max(out=degc[:], in0=degf[:], scalar1=1.0)
            dsq = wp.tile([128, G_GROUPS], F32, tag="deg")
            nc.scalar.activation(out=dsq[:], in_=degc[:],
                                 func=mybir.ActivationFunctionType.Sqrt)
            drc = wp.tile([128, G_GROUPS], F32, tag="deg")
            nc.vector.reciprocal(out=drc[:], in_=dsq[:])
            nc.vector.tensor_mul(out=dis[:], in0=drc[:], in1=mdeg[:])
            dis2 = lp.tile([128, G_GROUPS], F32)
            nc.vector.tensor_mul(out=dis2[:], in0=dis[:], in1=dis[:])

            # ---- z0 slice + out_acc init (hi/lo bf16 split of z)
            NSGL = (G_GROUPS + SUPER - 1) // SUPER
            for sg in range(NSGL):
                g0 = sg * SUPER
                W = min(SUPER, G_GROUPS - g0)
                et = wp.tile([128, SUPER, D], F32, tag="emb")
                nc.sync.dma_start(
                    out=et[:, :W, :],
                    in_=emb_e[g0 * 128:(g0 + W) * 128, :].rearrange(
                        "(g p) w -> p g w", p=128))
                z032 = wp.tile([128, SUPER, D], F32, tag="z032")
                nc.vector.tensor_tensor(out=z032[:, :W, :], in0=et[:, :W, :],
                                        in1=_bc_last(dis[:, g0:g0 + W], 128, W, D),
                                        op=mul)
                zh = wp.tile([128, SUPER, D], BF16, tag="zh")
                nc.vector.tensor_copy(out=zh[:, :W, :], in_=z032[:, :W, :])
                hi32 = wp.tile([128, SUPER, D], F32, tag="hi32")
                nc.vector.tensor_copy(out=hi32[:, :W, :], in_=zh[:, :W, :])
                zl = wp.tile([128, SUPER, D], BF16, tag="zl")
                nc.vector.tensor_tensor(out=zl[:, :W, :], in0=z032[:, :W, :],
                                        in1=hi32[:, :W, :],
                                        op=mybir.AluOpType.subtract)
                nc.sync.dma_start(
                    out=zslice[0][g0 * 128:(g0 + W) * 128, 0:D].rearrange(
                        "(g p) w -> p g w", p=128),
                    in_=zh[:, :W, :])
                nc.sync.dma_start(
                    out=zslice[0][g0 * 128:(g0 + W) * 128, D:ZW].rearrange(
                        "(g p) w -> p g w", p=128),
                    in_=zl[:, :W, :])
                nc.vector.tensor_tensor(
                    out=out_acc[:, g0:g0 + W, :], in0=et[:, :W, :],
                    in1=alpha_sb[:, 0:1].to_broadcast([128, W, D]),
                    op=mul)
            nc.gpsimd.collective_compute(
                "AllGather", mybir.AluOpType.bypass, replica_groups=rg,
                ins=[zslice[0].ap().opt()], outs=[zfull[0].ap().opt()])

            # ---- propagation layers
            qrr = 0
            disal = lp.tile([128, NLAYERS, G_GROUPS], F32)
            for l in range(NLAYERS):
                nc.vector.tensor_scalar(out=disal[:, l, :], in0=dis[:],
                                        scalar1=alpha_sb[:, l + 1:l + 2],
                                        scalar2=None, op0=mul)
            for l in range(NLAYERS):
                zf = zfull[l]
                win_tile = None
                win_id = -1
                state = dict(sg=-1, psum=None, done=[], started=False)

                def flush_groups():
                    psum = state["psum"]
                    if not state["done"]:
                        return
                    g0 = min(state["done"])
                    W = len(state["done"])
                    assert sorted(state["done"]) == list(range(g0, g0 + W))
                    xs = wp.tile([128, SUPER, D], F32, tag="xs")
                    ps_hl = bass.AP(psum.tensor, psum.offset,
                                    [list(psum.ap[0])[:1] + [128],
                                     [ZW, W], [1, D], [D, 2]])
                    nc.vector.reduce_sum(xs[:, :W, :], ps_hl,
                                         axis=mybir.AxisListType.X)
                    if l < NLAYERS - 1:
                        zn32 = wp.tile([128, SUPER, D], F32, tag="zn32")
                        nc.vector.tensor_tensor(
                            out=zn32[:, :W, :], in0=xs[:, :W, :],
                            in1=_bc_last(dis2[:, g0:g0 + W], 128, W, D),
                            op=mul)
                        zh = wp.tile([128, SUPER, D], BF16, tag="zh")
                        nc.vector.tensor_copy(out=zh[:, :W, :], in_=zn32[:, :W, :])
                        hi32 = wp.tile([128, SUPER, D], F32, tag="hi32")
                        nc.vector.tensor_copy(out=hi32[:, :W, :], in_=zh[:, :W, :])
                        zl = wp.tile([128, SUPER, D], BF16, tag="zl")
                        nc.vector.tensor_tensor(
                            out=zl[:, :W, :], in0=zn32[:, :W, :],
                            in1=hi32[:, :W, :], op=mybir.AluOpType.subtract)
                        nc.sync.dma_start(
                            out=zslice[l + 1][g0 * 128:(g0 + W) * 128,
                                              0:D].rearrange(
                                "(g p) w -> p g w", p=128),
                            in_=zh[:, :W, :])
                        nc.sync.dma_start(
                            out=zslice[l + 1][g0 * 128:(g0 + W) * 128,
                                              D:ZW].rearrange(
                                "(g p) w -> p g w", p=128),
                            in_=zl[:, :W, :])
                    t2 = wp.tile([128, SUPER, D], F32, tag="t2")
                    nc.vector.tensor_tensor(
                        out=t2[:, :W, :], in0=xs[:, :W, :],
                        in1=_bc_last(disal[:, l, g0:g0 + W], 128, W, D),
                        op=mul)
                    nc.vector.tensor_add(out=out_acc[:, g0:g0 + W, :],
                                         in0=out_acc[:, g0:g0 + W, :],
                                         in1=t2[:, :W, :])
                    state["done"] = []

                for ins in instrs:
                    c, n, icol, soff, sg = (ins["c"], ins["n"], ins["icol"],
                                            ins["soff"], ins["sg"])
                    if sg != state["sg"]:
                        flush_groups()
                        state["psum"] = psp.tile([128, SUPER * ZW], F32, tag="acc", name="psacc")
                        state["sg"] = sg
                        state["started"] = False
                    psum = state["psum"]
                    nb = n // 128
                    w = icol // IDXW
                    if w != win_id:
                        win_tile = ip.tile([128, IDXW], I16, tag="idxw")
                        nc.sync.dma_start(
                            out=win_tile[:],
                            in_=idx_e[:, w * IDXW:(w + 1) * IDXW])
                        win_id = w
                    gt = gp.tile([128, INSTR_MAX // 128, ZW], BF16, tag="gat")
                    wc = icol % IDXW
                    nc.gpsimd.dma_gather(
                        out_ap=gt[:, :nb, :],
                        in_ap=zf[clo[c]:chi[c], :],
                        idxs_ap=win_tile[:, wc:wc + n // 16],
                        num_idxs=n,
                        num_idxs_reg=n,
                        elem_size=ZW,
                        single_packet=False,
                        queue_num=qrr % 4,
                    )
                    qrr += 1
                    col0 = soff // 128
                    for p0 in range(0, nb, 8):
                        pnb = min(8, nb - p0)
                        bt = bp.tile([128, 8, 128], BF16, tag="bmat")
                        in0 = dsel_sb[:, col0 + p0:col0 + p0 + pnb
                                      ].to_broadcast([128, pnb, 128])
                        in1 = _bc_mid(iota_sb[:], 128, pnb, 128)
                        nc.vector.tensor_tensor(out=bt[:, :pnb, :], in0=in0,
                                                in1=in1,
                                                op=mybir.AluOpType.is_equal)
                        for j in range(pnb):
                            jj = p0 + j
                            _, g, first, last = ins["blocks"][jj]
                            gl = g % SUPER
                            nc.tensor.matmul(
                                psum[:, gl * ZW:(gl + 1) * ZW],
                                bt[:, j, :], gt[:, jj, :],
                                start=not state["started"], stop=last,
                                skip_group_check=True)
                            state["started"] = True
                            if last:
                                state["done"].append(g)
                flush_groups()
                if l < NLAYERS - 1:
                    nc.gpsimd.collective_compute(
                        "AllGather", mybir.AluOpType.bypass, replica_groups=rg,
                        ins=[zslice[l + 1].ap().opt()],
                        outs=[zfull[l + 1].ap().opt()])


def _epilogue(nc, tc, meta, env):
    clo, chi = _chunk_bounds()
    mul = mybir.AluOpType.mult
    add = mybir.AluOpType.add
    out_acc = env["out_acc"]; dis = env["dis"]; scal_sb = env["scal_sb"]
    mitem_sb = env["mitem_sb"]; psp = env["psp"]
    oslice = env["oslice"]; ofull = env["ofull"]; rrow_d = env["rrow_d"]
    lab_e = env["lab_e"]; bwin_e = env["bwin_e"]; e8_e = env["e8_e"]
    beta_e = env["beta_e"]; osel_e = env["osel_e"]; out_e = env["out_e"]
    pp = env["pp"]
    rg = [list(range(NCORE))]
    with (
        tc.tile_pool(name="ep", bufs=1) as ep,
        tc.tile_pool(name="ew", bufs=3) as wp,
        tc.tile_pool(name="eg", bufs=2) as gp,
        tc.tile_pool(name="orow", bufs=2) as op,
    ):
            # ---- item normalization + out slice
            for g in range(G_GROUPS):
                sq = wp.tile([128, D], F32, tag="sq")
                nc.vector.tensor_mul(out=sq[:], in0=out_acc[:, g, :],
                                     in1=out_acc[:, g, :])
                ss = wp.tile([128, 1], F32, tag="ss")
                nc.vector.reduce_sum(ss[:], sq[:], axis=mybir.AxisListType.X)
                nrm = wp.tile([128, 1], F32, tag="ss")
                nc.scalar.activation(out=nrm[:], in_=ss[:],
                                     func=mybir.ActivationFunctionType.Sqrt)
                nc.vector.tensor_scalar_max(out=nrm[:], in0=nrm[:], scalar1=EPS)
                rec = wp.tile([128, 1], F32, tag="ss")
                nc.vector.reciprocal(out=rec[:], in_=nrm[:])
                fac = wp.tile([128, 1], F32, tag="ss")
                nc.vector.tensor_scalar(out=fac[:], in0=rec[:],
                                        scalar1=scal_sb[:, 0:1], scalar2=-1.0,
                                        op0=mul, op1=add)
                nc.vector.tensor_scalar(out=fac[:], in0=fac[:],
                                        scalar1=mitem_sb[:, g:g + 1], scalar2=1.0,
                                        op0=mul, op1=add)
                on = wp.tile([128, D], F32, tag="on")
                nc.vector.tensor_scalar(out=on[:], in0=out_acc[:, g, :],
                                        scalar1=fac[:, 0:1], scalar2=None,
                                        op0=mul)
                nc.sync.dma_start(out=oslice[g * 128:(g + 1) * 128, :], in_=on[:])
            nc.gpsimd.collective_compute(
                "AllGather", mybir.AluOpType.bypass, replica_groups=rg,
                ins=[oslice.ap().opt()], outs=[ofull.ap().opt()])

            # ---- label gathers (16384 slots, 5 chunk passes summed)
            lab_sb = ep.tile([128, NCHUNK * 1024], I16, tag="lab")
            nc.sync.dma_start(out=lab_sb[:], in_=lab_e[:, :])
            acc_lab = pp.tile([128, 128, D], F32, tag="out_acc")
            for c in range(NCHUNK):
                for half in range(2):
                    lt = gp.tile([128, 64, D], F32, tag="labg")
                    for piece in range(8):
                        pc = half * 8 + piece
                        nc.gpsimd.dma_gather(
                            out_ap=lt[:, piece * 8:(piece + 1) * 8, :],
                            in_ap=ofull[clo[c]:chi[c], :],
                            idxs_ap=lab_sb[:, c * 1024 + pc * 64:
                                           c * 1024 + (pc + 1) * 64],
                            num_idxs=1024,
                            num_idxs_reg=1024,
                            elem_size=D,
                            single_packet=True,
                            queue_num=pc % 4,
                        )
                    dst = acc_lab[:, half * 64:(half + 1) * 64, :]
                    if c == 0:
                        nc.vector.tensor_copy(out=dst, in_=lt[:])
                    else:
                        nc.vector.tensor_add(out=dst, in0=dst, in1=lt[:])

            # rank[i] at [i%128, i//128]; src slots g 0..63, dst slots g 64..127
            rmul = ep.tile([128, 64, D], F32, tag="e16a")
            nc.vector.tensor_mul(out=rmul[:], in0=acc_lab[:, :64, :],
                                 in1=acc_lab[:, 64:, :])
            rank = ep.tile([128, 64], F32, tag="rank")
            nc.vector.reduce_sum(rank[:], rmul[:], axis=mybir.AxisListType.X)

            # ---- beta windows
            bwin_sb = ep.tile([128, 512], I16, tag="bwin")
            nc.sync.dma_start(out=bwin_sb[:], in_=bwin_e[:, :])
            e8_sb = ep.tile([128, 64, D], F32, tag="e16b")
            nc.sync.dma_start(out=e8_sb[:],
                              in_=e8_e[:, :].rearrange("p (g d) -> p g d", d=D))
            bw = ep.tile([128, 64, D], F32, tag="e16c")
            for piece in range(8):
                nc.gpsimd.dma_gather(
                    out_ap=bw[:, piece * 8:(piece + 1) * 8, :],
                    in_ap=beta_e[:, :],
                    idxs_ap=bwin_sb[:, piece * 64:(piece + 1) * 64],
                    num_idxs=1024,
                    num_idxs_reg=1024,
                    elem_size=D,
                    single_packet=True,
                    queue_num=piece % 4,
                )
            bsel = ep.tile([128, 64, D], F32, tag="e16a")
            nc.vector.tensor_mul(out=bsel[:], in0=bw[:], in1=e8_sb[:])
            betad = ep.tile([128, 64], F32, tag="betad")
            nc.vector.reduce_sum(betad[:], bsel[:], axis=mybir.AxisListType.X)

            # ---- broadcast rank to all partitions as a free-dim row
            ident = ep.tile([128, 128], F32, tag="ident")
            make_identity(nc, ident[:])
            tps = psp.tile([64, 128], F32, tag="tp")
            nc.tensor.transpose(out=tps[:], in_=rank[:, :], identity=ident[:])
            tr = ep.tile([64, 128], F32, tag="tr")
            nc.vector.tensor_copy(out=tr[:], in_=tps[:])
            nc.sync.dma_start(
                out=rrow_d[0, :].rearrange("(a b) -> a b", b=128), in_=tr[:])
            rbc = ep.tile([128, NLAB], F32, tag="rbc")
            nc.sync.dma_start(out=rbc[:],
                              in_=rrow_d[0:1, :].to_broadcast([128, NLAB]))

            # ---- outer sums, rows sharded by osel
            osel_sb = ep.tile([128, 8, 64], F32, tag="osel")
            nc.sync.dma_start(out=osel_sb[:],
                              in_=osel_e[:, :].rearrange("p (t g) -> p t g", g=64))
            for t in range(4):
                for pn in range(2):
                    sel = wp.tile([128, 64], F32, tag="osel_t")
                    nc.vector.tensor_mul(out=sel[:], in0=betad[:],
                                         in1=osel_sb[:, pn * 4 + t, :])
                    bval = wp.tile([128, 1], F32, tag="bval")
                    nc.vector.reduce_sum(bval[:], sel[:], axis=mybir.AxisListType.X)
                    for half in range(2):
                        orow = op.tile([128, 2048], F32, tag="orow")
                        nc.vector.tensor_scalar_add(
                            out=orow[:],
                            in0=rbc[:, pn * 4096 + half * 2048:
                                    pn * 4096 + (half + 1) * 2048],
                            scalar1=bval[:, 0:1])
                        nc.sync.dma_start(
                            out=out_e[pn, t * 128:(t + 1) * 128,
                                      half * 2048:(half + 1) * 2048],
                            in_=orow[:])


# ---------------------------------------------------------------- entry
def kernel(emb_weight, beta_weight, alpha, edge_index, edge_label_index,
           num_users, num_items, scaling_factor):
    in_maps, meta = _prep(emb_weight, beta_weight, alpha, edge_index,
                          edge_label_index, num_users, scaling_factor)
    nc = _build(meta)
    nc.finalize()
    res = run_bass_kernel_spmd(nc, in_maps, list(range(NCORE)))
    pos = np.concatenate([res.results[k]["out"][0] for k in range(NCORE)], axis=0)
    neg = np.concatenate([res.results[k]["out"][1] for k in range(NCORE)], axis=0)
    return pos, neg
